# revision 3
# baseline (speedup 1.0000x reference)
"""ConflictAwareResidualRouter Trainium2 Bass kernel (v3, 16-bit data path).

Shards the B*S=8192 tokens across 8 NeuronCores (1024 tokens each).
Gate/reliability weights are replicated; the routed weighted residual sum is
purely local per token.

The baseline (v2, all-fp32) was HBM-bound: 87 MB/core at ~302 GB/s => 288 us.
v3 halves the big streams to 16-bit (measured on this problem's inputs:
1 top-2 selection flip out of 8192 tokens, full-pipeline rel_l2 ~ 8e-3 vs
the 2e-2 gate):
  - h            -> fp16 (pre-transposed chunk layout, host-side)
  - static_delta -> fp16
  - adapter_residuals -> fp16 (indirect-gathered rows of 8KB)
  - out          -> fp16, upcast to fp32 on host
Gate logits stay effectively fp32: the fused [Wp | W1h] operand is fp16 for
Wp (feat path, selection-insensitive) plus a split-fp16 W1 (hi + lo*2048, the
lo scaled to avoid fp16 subnormals); PSUM accumulates in fp32 and the halves
are merged with one DVE op. All small-head matmuls (Wh, Wx, W2) remain fp32.

Per-core pipeline (token tiles of 128):
  1. psum[t,320] = sum_c ht_c.T-chunks @ [Wp|W1hi|W1lo*2048]_c (32 fp16 matmuls)
  2. feat=relu(psum[:,0:64]); rel=sigmoid(feat@Wh); extra matmul adds
     [rel,conflict] @ W1x into psum[:,64:192]; hid=relu(hi + lo/2048)
  3. logits[t,6] = hid @ W2 (via one PE transpose of hid)
  4. top-2 mask over adapter logits (threshold trick) + softmax (DVE/ACT)
  5. acc = g1*static + ga*r0 + gb*r1 over d-chunks (ACT scale-copy + DVE
     fused scalar_tensor_tensor chain, fp16 tensors with fp32 scalars)

Biases are asserted zero (spec fill=zeros) and skipped on device.
"""

import numpy as np

import concourse.bass as bass
import concourse.mybir as mybir
import concourse.tile as tile
from concourse import bacc
from concourse.masks import make_identity

F32 = mybir.dt.float32
F16 = mybir.dt.float16
I32 = mybir.dt.int32
AF = mybir.ActivationFunctionType
OP = mybir.AluOpType

N_CORES = 8
B, S, D = 4, 2048, 4096
N_TOK_FULL = B * S
TPC = N_TOK_FULL // N_CORES  # tokens per core
P = 128                      # token tile size / partitions
DCHUNK = 2048                # d chunk for the weighted-sum stage
NA = 4                       # adapters
RH = 64                      # reliability hidden
H = 128                      # gate hidden
NCH = RH + H                 # hi half width (feat | hid)
WLO = H                      # lo half width (hid correction)
NW = NCH + WLO               # fused matmul output width, 320
NC_CHOICES = 6               # [base, static, a0..a3]
KC = D // P                  # 32 contraction chunks
NEG_BIG = -1.0e30
LO_SCALE = 2048.0            # W1 lo half is pre-scaled by this (fp16 subnormals)


def build_nc(n_tok=TPC):
    from contextlib import ExitStack

    assert n_tok % P == 0
    n_tiles = n_tok // P
    nc = bacc.Bacc("TRN2", target_bir_lowering=False, debug=False)

    # ht[tile, d_in_chunk(128), chunk(32), tok(128)] — host-pretransposed h
    ht_d = nc.dram_tensor("ht", [n_tiles, P, KC, P], F16, kind="ExternalInput")
    st_d = nc.dram_tensor("static", [n_tok, D], F16, kind="ExternalInput")
    # row (a*n_tok + t) = adapter a's residual for token t; gathered by top-2
    res_d = nc.dram_tensor("res", [NA * n_tok, D], F16, kind="ExternalInput")
    cf_d = nc.dram_tensor("conflict", [n_tok, NA], F32, kind="ExternalInput")
    # pidx[p] = p (partition index), used to build gather row indices
    pidx_d = nc.dram_tensor("pidx", [P, 1], F32, kind="ExternalInput")
    iota4_d = nc.dram_tensor("iota4", [P, NA], F32, kind="ExternalInput")
    # wcat[d_in_chunk(128), chunk(32), out(320)] — host-fused [Wp|W1hi|W1lo*2048]
    wcat_d = nc.dram_tensor("wcat", [P, KC, NW], F16, kind="ExternalInput")
    wx_d = nc.dram_tensor("wx", [2 * NA, H], F32, kind="ExternalInput")
    wh_d = nc.dram_tensor("wh", [RH, NA], F32, kind="ExternalInput")
    w2_d = nc.dram_tensor("w2", [H, NC_CHOICES], F32, kind="ExternalInput")
    out_d = nc.dram_tensor("out", [n_tok, D], F16, kind="ExternalOutput")

    with tile.TileContext(nc) as tc, ExitStack() as ctx:
        const = ctx.enter_context(tc.tile_pool(name="const", bufs=1))
        ht_pool = ctx.enter_context(tc.tile_pool(name="ht", bufs=2))
        small = ctx.enter_context(tc.tile_pool(name="small", bufs=2))
        gpool = ctx.enter_context(tc.tile_pool(name="gates", bufs=3))
        chunk = ctx.enter_context(tc.tile_pool(name="chunk", bufs=6))
        rpool = ctx.enter_context(tc.tile_pool(name="rsel", bufs=3))
        accp = ctx.enter_context(tc.tile_pool(name="acc", bufs=4))
        ps_main = ctx.enter_context(tc.tile_pool(name="ps_main", bufs=2, space="PSUM"))
        ps_small = ctx.enter_context(tc.tile_pool(name="ps_small", bufs=2, space="PSUM"))

        # --- constants ---
        ident = const.tile([P, P], F32)
        make_identity(nc, ident[:])
        wcat_sb = const.tile([P, KC, NW], F16)
        nc.sync.dma_start(wcat_sb[:], wcat_d[:])
        wx_sb = const.tile([P, H], F32)  # rows 0..7 = W1[4096:4104], rest 0
        nc.vector.memset(wx_sb[:], 0.0)
        nc.sync.dma_start(wx_sb[0 : 2 * NA, :], wx_d[:])
        wh_sb = const.tile([P, NA], F32)  # rows 0..63 = Wh, rest 0
        nc.vector.memset(wh_sb[:], 0.0)
        nc.sync.dma_start(wh_sb[0:RH, :], wh_d[:])
        w2_sb = const.tile([P, NC_CHOICES], F32)
        nc.sync.dma_start(w2_sb[:], w2_d[:])
        pidx_sb = const.tile([P, 1], F32)
        nc.sync.dma_start(pidx_sb[:], pidx_d[:])
        iota4_sb = const.tile([P, NA], F32)
        nc.sync.dma_start(iota4_sb[:], iota4_d[:])

        for tk in range(n_tiles):
            tok = slice(tk * P, (tk + 1) * P)

            # ---- fused feat|hid|hid_lo matmul over 32 d-chunks ----
            ht_sb = ht_pool.tile([P, KC, P], F16, tag="ht")
            nc.sync.dma_start(ht_sb[:], ht_d[tk])
            ps1 = ps_main.tile([P, NW], F32, tag="ps1")
            for c in range(KC):
                nc.tensor.matmul(
                    ps1[:], ht_sb[:, c, :], wcat_sb[:, c, :],
                    start=(c == 0), stop=False, skip_group_check=True,
                )

            # ---- reliability head: rel = sigmoid(feat @ Wh) ----
            feat_sb = small.tile([P, RH], F32, tag="feat")
            nc.scalar.activation(feat_sb[:], ps1[:, 0:RH], AF.Relu)
            pft = ps_small.tile([RH, P], F32, tag="ps_small")
            nc.tensor.transpose(pft[:], feat_sb[:], ident[:])
            featT = small.tile([P, P], F32, tag="featT")  # rows 64.. stay 0
            nc.gpsimd.memset(featT[:], 0.0)
            nc.vector.tensor_copy(featT[0:RH, :], pft[:])
            prel = ps_small.tile([P, NA], F32, tag="ps_small")
            nc.tensor.matmul(prel[:], featT[:], wh_sb[:], start=True, stop=True)

            # ---- extra gate features [rel | conflict] -> [t, 8] ----
            ex_sb = small.tile([P, 2 * NA], F32, tag="ex")
            nc.scalar.activation(ex_sb[:, 0:NA], prel[:], AF.Sigmoid)
            nc.sync.dma_start(ex_sb[:, NA : 2 * NA], cf_d[tok, :])
            pxt = ps_small.tile([2 * NA, P], F32, tag="ps_small")
            nc.tensor.transpose(pxt[:], ex_sb[:], ident[:])
            exT = small.tile([P, P], F32, tag="exT")  # rows 8.. stay 0
            nc.gpsimd.memset(exT[:], 0.0)
            nc.vector.tensor_copy(exT[0 : 2 * NA, :], pxt[:])

            # ---- close hid accumulation: += exT.T @ W1x ----
            nc.tensor.matmul(
                ps1[:, RH:NCH], exT[:], wx_sb[:],
                start=False, stop=True, skip_group_check=True,
            )
            # hid = relu(hi + lo/LO_SCALE); DVE may read only one PSUM operand,
            # so ACT first drains the scaled lo half to SBUF
            hidlo = small.tile([P, H], F32, tag="hidlo")
            nc.scalar.activation(hidlo[:], ps1[:, NCH:NW], AF.Copy, scale=1.0 / LO_SCALE)
            hidp = small.tile([P, H], F32, tag="hidp")
            nc.vector.tensor_tensor(hidp[:], hidlo[:], ps1[:, RH:NCH], op=OP.add)
            hid_sb = small.tile([P, H], F32, tag="hid")
            nc.scalar.activation(hid_sb[:], hidp[:], AF.Relu)

            # ---- logits [t, 6] = hid @ W2 ----
            pht = ps_small.tile([H, P], F32, tag="ps_small")
            nc.tensor.transpose(pht[:], hid_sb[:], ident[:])
            hidT = small.tile([P, P], F32, tag="hidT")
            nc.vector.tensor_copy(hidT[:], pht[:])
            plg = ps_small.tile([P, NC_CHOICES], F32, tag="ps_small")
            nc.tensor.matmul(plg[:], hidT[:], w2_sb[:], start=True, stop=True)
            lg = gpool.tile([P, NC_CHOICES], F32, tag="lg")
            nc.vector.tensor_copy(lg[:], plg[:])

            # ---- top-2 over adapter logits + softmax over 6 ----
            ad = lg[:, 2:6]
            m1 = gpool.tile([P, 1], F32, tag="m1")
            nc.vector.tensor_reduce(m1[:], ad, axis=mybir.AxisListType.X, op=OP.max)
            eqm = gpool.tile([P, NA], F32, tag="eqm")
            nc.vector.tensor_scalar(eqm[:], ad, m1[:, 0:1], None, op0=OP.is_ge)
            tmp4 = gpool.tile([P, NA], F32, tag="tmp4")
            nc.vector.scalar_tensor_tensor(
                tmp4[:], eqm[:], NEG_BIG, ad, op0=OP.mult, op1=OP.add
            )
            m2 = gpool.tile([P, 1], F32, tag="m2")
            nc.vector.tensor_reduce(m2[:], tmp4[:], axis=mybir.AxisListType.X, op=OP.max)
            keep = gpool.tile([P, NA], F32, tag="keep")
            nc.vector.tensor_scalar(keep[:], ad, m2[:, 0:1], None, op0=OP.is_ge)
            negm = gpool.tile([P, NA], F32, tag="negm")
            nc.vector.tensor_scalar(
                negm[:], keep[:], -NEG_BIG, NEG_BIG, op0=OP.mult, op1=OP.add
            )
            kept = gpool.tile([P, NA], F32, tag="kept")
            nc.vector.tensor_tensor(kept[:], ad, keep[:], op=OP.mult)
            nc.vector.tensor_tensor(lg[:, 2:6], kept[:], negm[:], op=OP.add)
            nmx = gpool.tile([P, 1], F32, tag="nmx")
            nc.vector.tensor_reduce(
                nmx[:], lg[:], axis=mybir.AxisListType.X, op=OP.max, negate=True
            )
            ex6 = gpool.tile([P, NC_CHOICES], F32, tag="ex6")
            nc.scalar.activation(ex6[:], lg[:], AF.Exp, bias=nmx[:, 0:1], scale=1.0)
            ssum = gpool.tile([P, 1], F32, tag="ssum")
            nc.vector.tensor_reduce(ssum[:], ex6[:], axis=mybir.AxisListType.X, op=OP.add)
            rinv = gpool.tile([P, 1], F32, tag="rinv")
            nc.vector.reciprocal(rinv[:], ssum[:])
            g = gpool.tile([P, NC_CHOICES], F32, tag="g")
            nc.vector.tensor_scalar(g[:], ex6[:], rinv[:, 0:1], None, op0=OP.mult)

            # ---- top-2 selection: adapter ids + gate values per token ----
            selm1 = gpool.tile([P, NA], F32, tag="selm1")  # 2nd-place one-hot
            nc.vector.tensor_tensor(selm1[:], keep[:], eqm[:], op=OP.subtract)
            t0 = gpool.tile([P, NA], F32, tag="t0")
            nc.vector.tensor_tensor(t0[:], eqm[:], iota4_sb[:], op=OP.mult)
            sel0 = gpool.tile([P, 1], F32, tag="sel0")
            nc.vector.tensor_reduce(sel0[:], t0[:], axis=mybir.AxisListType.X, op=OP.add)
            t1 = gpool.tile([P, NA], F32, tag="t1")
            nc.vector.tensor_tensor(t1[:], selm1[:], iota4_sb[:], op=OP.mult)
            sel1 = gpool.tile([P, 1], F32, tag="sel1")
            nc.vector.tensor_reduce(sel1[:], t1[:], axis=mybir.AxisListType.X, op=OP.add)
            ga_t = gpool.tile([P, NA], F32, tag="ga_t")
            nc.vector.tensor_tensor(ga_t[:], g[:, 2:6], eqm[:], op=OP.mult)
            ga = gpool.tile([P, 1], F32, tag="ga")
            nc.vector.tensor_reduce(ga[:], ga_t[:], axis=mybir.AxisListType.X, op=OP.add)
            gb_t = gpool.tile([P, NA], F32, tag="gb_t")
            nc.vector.tensor_tensor(gb_t[:], g[:, 2:6], selm1[:], op=OP.mult)
            gb = gpool.tile([P, 1], F32, tag="gb")
            nc.vector.tensor_reduce(gb[:], gb_t[:], axis=mybir.AxisListType.X, op=OP.add)
            # gather row index: idx_s = sel_s * n_tok + tk*P + p
            pb = gpool.tile([P, 1], F32, tag="pb")
            nc.vector.tensor_scalar(pb[:], pidx_sb[:], float(tk * P), None, op0=OP.add)
            max_row = float(NA * n_tok - 1)
            idx0f = gpool.tile([P, 1], F32, tag="idx0f")
            nc.vector.scalar_tensor_tensor(
                idx0f[:], sel0[:], float(n_tok), pb[:], op0=OP.mult, op1=OP.add
            )
            nc.vector.tensor_scalar(idx0f[:], idx0f[:], max_row, None, op0=OP.min)
            idx0 = gpool.tile([P, 1], I32, tag="idx0")
            nc.vector.tensor_copy(idx0[:], idx0f[:])
            idx1f = gpool.tile([P, 1], F32, tag="idx1f")
            nc.vector.scalar_tensor_tensor(
                idx1f[:], sel1[:], float(n_tok), pb[:], op0=OP.mult, op1=OP.add
            )
            nc.vector.tensor_scalar(idx1f[:], idx1f[:], max_row, None, op0=OP.min)
            idx1 = gpool.tile([P, 1], I32, tag="idx1")
            nc.vector.tensor_copy(idx1[:], idx1f[:])

            # ---- gather the two selected residual rows (8KB each) ----
            r0 = rpool.tile([P, D], F16, tag="r0")
            nc.gpsimd.indirect_dma_start(
                out=r0[:], out_offset=None, in_=res_d[:],
                in_offset=bass.IndirectOffsetOnAxis(ap=idx0[:, 0:1], axis=0),
            )
            r1 = rpool.tile([P, D], F16, tag="r1")
            nc.gpsimd.indirect_dma_start(
                out=r1[:], out_offset=None, in_=res_d[:],
                in_offset=bass.IndirectOffsetOnAxis(ap=idx1[:, 0:1], axis=0),
            )

            # ---- weighted residual sum, d in chunks ----
            for dc in range(D // DCHUNK):
                dsl = slice(dc * DCHUNK, (dc + 1) * DCHUNK)
                st_sb = chunk.tile([P, DCHUNK], F16, tag="st")
                nc.sync.dma_start(st_sb[:], st_d[tok, dsl])
                acc = accp.tile([P, DCHUNK], F16, tag="acc")
                nc.scalar.activation(acc[:], st_sb[:], AF.Copy, scale=g[:, 1:2])
                nc.vector.scalar_tensor_tensor(
                    acc[:], r0[:, dsl], ga[:, 0:1], acc[:], op0=OP.mult, op1=OP.add
                )
                nc.vector.scalar_tensor_tensor(
                    acc[:], r1[:, dsl], gb[:, 0:1], acc[:], op0=OP.mult, op1=OP.add
                )
                nc.scalar.dma_start(out_d[tok, dsl], acc[:])

    nc.compile()
    return nc


_NC_CACHE = {}


def _get_nc(n_tok=TPC):
    if n_tok not in _NC_CACHE:
        _NC_CACHE[n_tok] = build_nc(n_tok)
    return _NC_CACHE[n_tok]


def _prep_ht(h_core):
    """[n_tok, D] fp16 -> [n_tiles, 128, 32, 128] pre-transposed chunk layout."""
    n_tok = h_core.shape[0]
    n_tiles = n_tok // P
    # ht[tk, p, c, t] = h[tk*128 + t, c*128 + p]
    v = h_core.reshape(n_tiles, P, KC, P)  # [tk, t, c, p]
    return np.ascontiguousarray(v.transpose(0, 3, 2, 1))


def make_in_maps(inputs, n_cores=N_CORES, n_tok=TPC):
    f = np.float32
    h16 = np.asarray(inputs["h"], dtype=f).reshape(N_TOK_FULL, D).astype(np.float16)
    st = np.asarray(inputs["static_delta"], dtype=f).reshape(N_TOK_FULL, D)
    st16 = st.astype(np.float16)
    res = np.asarray(inputs["adapter_residuals"], dtype=f).reshape(NA, N_TOK_FULL, D)
    res16 = res.astype(np.float16)
    cf = np.asarray(inputs["conflict_scores"], dtype=f).reshape(N_TOK_FULL, NA)
    for bname in ("rel_proj_b", "rel_heads_b", "gate_b1", "gate_b2"):
        bv = np.asarray(inputs[bname])
        assert not bv.any(), f"{bname} expected all-zero (spec fill=zeros)"
    wp = np.asarray(inputs["rel_proj_w"], dtype=f)
    w1 = np.asarray(inputs["gate_w1"], dtype=f)
    whi = np.concatenate([wp, w1[0:D]], axis=1).astype(np.float16)  # [4096, 192]
    w1h16 = w1[0:D].astype(np.float16)
    wlo = ((w1[0:D] - w1h16.astype(f)) * LO_SCALE).astype(np.float16)  # [4096, 128]
    wcat = np.concatenate(
        [whi.reshape(KC, P, NCH), wlo.reshape(KC, P, WLO)], axis=2
    ).transpose(1, 0, 2)  # [128, 32, 320]
    shared = {
        "wcat": np.ascontiguousarray(wcat),
        "wx": np.ascontiguousarray(w1[D : D + 2 * NA]),
        "wh": np.ascontiguousarray(inputs["rel_heads_w"], dtype=f),
        "w2": np.ascontiguousarray(inputs["gate_w2"], dtype=f),
        "pidx": np.arange(P, dtype=f).reshape(P, 1),
        "iota4": np.tile(np.arange(NA, dtype=f), (P, 1)),
    }
    in_maps = []
    for c in range(n_cores):
        sl = slice(c * n_tok, (c + 1) * n_tok)
        in_maps.append(
            {
                "ht": _prep_ht(h16[sl]),
                "static": np.ascontiguousarray(st16[sl]),
                "res": np.ascontiguousarray(res16[:, sl]).reshape(NA * n_tok, D),
                "conflict": np.ascontiguousarray(cf[sl]),
                **shared,
            }
        )
    return in_maps


def _ensure_axon_hooks_module():
    """The agent image's antenv lacks axon_hooks; bass_utils imports it when
    tracing is requested (BASS_TRACE=1). Register a stub so a traced run
    degrades to untraced instead of crashing."""
    import sys
    import types

    try:
        import antenv.axon_hooks  # noqa: F401
    except ImportError:
        mod = types.ModuleType("antenv.axon_hooks")
        mod.get_axon_ntff_profile_hook = lambda: None
        mod.set_axon_ntff_profile_hook = lambda h: None
        sys.modules["antenv.axon_hooks"] = mod


def kernel(**inputs) -> np.ndarray:
    _ensure_axon_hooks_module()
    from concourse.bass_utils import run_bass_kernel_spmd

    nc = _get_nc(TPC)
    in_maps = make_in_maps(inputs)
    res = run_bass_kernel_spmd(nc, in_maps, core_ids=list(range(N_CORES)))
    out = np.concatenate([r["out"] for r in res.results], axis=0)
    return out.astype(np.float32).reshape(B, S, D)
